# revision 40
# baseline (speedup 1.0000x reference)
"""Trainium2 Bass kernel for nn_BestModel5 (dual-GRU encoder + BxB pair classifier).

Sharding (8 cores): cores 0-3 query-GRU batch shards of 64; cores 4-7 reply-GRU.
Classifier sharded 8-way over the 256 query rows (32 i-rows/core).

GRU: the x-contribution (incl. bias + length mask via extra contraction rows)
is matmul'd straight from the embedded sequence into the step's PSUM bank two
steps at a time, so the recurrent critical chain is just
  sig_r -> rh=r*h -> cand MMs -> tanh -> oc=(1-z)*c -> oc gate MMs -> sig_r
with h' = z*h + (1-z)*c; the z*h gate matmuls run off-chain right after
sigmoid(z) lands.  Sigmoid/tanh read PSUM directly.
"""

import numpy as np
import ml_dtypes

BF16 = ml_dtypes.bfloat16

V, E, H, B, T = 100000, 256, 256, 256, 40
D_HID, D_OUT = 256, 2
NCORES = 8
BSH = 64          # batch rows per GRU shard
NSH = 4           # GRU batch shards per encoder
BT = BSH * T      # 2560 columns of XembT per core
IBLK = B // NCORES  # 32 classifier i-rows per core
NPAIR = T // 2    # step pairs sharing one PSUM bank

_cache = {}


def _build(sim_gelu=False):
    """Build + compile the SPMD Bass program once."""
    import concourse.bacc as bacc
    import concourse.tile as tile
    import concourse.mybir as mybir

    f32 = mybir.dt.float32
    bf16 = mybir.dt.bfloat16
    AF = mybir.ActivationFunctionType

    nc = bacc.Bacc("TRN2", target_bir_lowering=False, debug=False, num_devices=NCORES)

    def din(name, shape, dt):
        return nc.dram_tensor(name, shape, dt, kind="ExternalInput").ap()

    # per-core inputs (content differs per core; shapes identical)
    xembT = din("xembT", [E + 2, BT], bf16)      # rows: emb dims, ones, len-mask
    whg = din("whg", [H, 2 * H], bf16)           # Wg[E:E+H, :]
    wxg = din("wxg", [E + 2, 2 * H], bf16)       # Wg[:E, :]; bg; 30-on-z mask row
    wch = din("wch", [H, H], bf16)               # Wc[E:E+H, :]
    wxc = din("wxc", [E + 1, H], bf16)           # Wc[:E, :]; bc row
    ident = din("ident", [128, 128], bf16)       # identity for r1 PSUM adds
    w1q = din("w1q", [H, D_HID], bf16)           # W1[:256]
    w1r = din("w1r", [H, D_HID], bf16)           # W1[257:513]
    wdt = din("wdt", [1, IBLK // 2 * D_HID], bf16)  # W1[256] tiled 16x
    rhsb = din("rhsb", [4, IBLK * B], bf16)      # [0;ones|0;0|0;0;ones] pattern
    b1 = din("b1", [D_HID], f32)
    w2 = din("w2", [D_HID, D_OUT], bf16)

    # out[2*sl + o, 512*ib + 256*ii + j] for pair-block pr = 3*ib + sl
    out = nc.dram_tensor("out", [6, 6 * 512], f32,
                         kind="ExternalOutput").ap()
    hdbg = nc.dram_tensor("hdbg", [128, 128], bf16,
                          kind="ExternalOutput").ap()

    with tile.TileContext(nc) as tc:
        with (
            tc.tile_pool(name="persist", bufs=1) as pp,
            tc.tile_pool(name="dram", bufs=1, space="DRAM") as dramp,
        ):
            # ---- load weights/constants to SBUF ----
            xT = [pp.tile([128, BT], bf16, tag=f"xT{k}", name=f"xT{k}") for k in range(2)]
            # ones/len-mask rows zero-padded to K=128 (small-K matmul
            # accumulations silently drop contributions on this stack)
            xE = pp.tile([128, BT], bf16, tag="xE", name="xE")
            nc.vector.memset(xE[:], 0.0)
            dma_eng = [nc.sync, nc.scalar, nc.gpsimd]
            for n in range(5):
                cs = slice(512 * n, 512 * n + 512)
                dma_eng[n % 3].dma_start(xT[0][:, cs], xembT[0:128, cs])
                dma_eng[(n + 1) % 3].dma_start(xT[1][:, cs], xembT[128:256, cs])
                dma_eng[(n + 2) % 3].dma_start(xE[0:2, cs], xembT[256:258, cs])

            whg_s = [pp.tile([128, 2 * H], bf16, tag=f"whg{k}", name=f"whg{k}") for k in range(2)]
            nc.sync.dma_start(whg_s[0][:], whg[0:128, :])
            nc.sync.dma_start(whg_s[1][:], whg[128:256, :])
            wxg_s = [pp.tile([128, 2 * H], bf16, tag=f"wxg{k}", name=f"wxg{k}") for k in range(2)]
            nc.sync.dma_start(wxg_s[0][:], wxg[0:128, :])
            nc.sync.dma_start(wxg_s[1][:], wxg[128:256, :])
            wxg_e = pp.tile([128, 2 * H], bf16, tag="wxge", name="wxge")
            nc.gpsimd.memset(wxg_e[:], 0.0)
            nc.sync.dma_start(wxg_e[0:2, :], wxg[256:258, :])

            wch_s = [pp.tile([128, H], bf16, tag=f"wch{k}", name=f"wch{k}") for k in range(2)]
            nc.sync.dma_start(wch_s[0][:], wch[0:128, :])
            nc.sync.dma_start(wch_s[1][:], wch[128:256, :])
            wxc_s = [pp.tile([128, H], bf16, tag=f"wxc{k}", name=f"wxc{k}") for k in range(2)]
            nc.sync.dma_start(wxc_s[0][:], wxc[0:128, :])
            nc.sync.dma_start(wxc_s[1][:], wxc[128:256, :])
            wxc_e = pp.tile([128, H], bf16, tag="wxce", name="wxce")
            nc.gpsimd.memset(wxc_e[:], 0.0)
            nc.sync.dma_start(wxc_e[0:1, :], wxc[256:257, :])

            id_bf = pp.tile([128, 128], bf16, tag="idbf", name="idbf")
            nc.scalar.dma_start(id_bf[:], ident[:])

            w1q_s = [pp.tile([128, D_HID], bf16, tag=f"w1q{k}", name=f"w1q{k}") for k in range(2)]
            nc.sync.dma_start(w1q_s[0][:], w1q[0:128, :])
            nc.sync.dma_start(w1q_s[1][:], w1q[128:256, :])
            w1r_s = [pp.tile([128, D_HID], bf16, tag=f"w1r{k}", name=f"w1r{k}") for k in range(2)]
            nc.sync.dma_start(w1r_s[0][:], w1r[0:128, :])
            nc.sync.dma_start(w1r_s[1][:], w1r[128:256, :])

            b1_s = pp.tile([128, 2], f32, tag="b1", name="b1")
            nc.sync.dma_start(b1_s[:], b1.rearrange("(m p) -> p m", p=128))
            w2_s = [pp.tile([128, D_OUT], bf16, tag=f"w2{k}", name=f"w2{k}") for k in range(2)]
            nc.sync.dma_start(w2_s[0][:], w2[0:128, :])
            nc.sync.dma_start(w2_s[1][:], w2[128:256, :])

            # collective tiles + warmup AllGather (absorbs cc cold-start)
            ag_in = dramp.tile([128, 128], bf16, tag="agin", name="agin")
            ag_out = dramp.tile([NCORES, 128, 128], bf16, tag="agout", name="agout")
            zer_bf = pp.tile([128, 128], bf16, tag="zer", name="zer")
            nc.gpsimd.memset(zer_bf[:], 0.0)
            zer512 = pp.tile([128, 512], bf16, tag="zer512", name="zer512")
            nc.gpsimd.memset(zer512[:], 0.0)
            nc.gpsimd.dma_start(ag_in[:], zer_bf[:])
            nc.gpsimd.collective_compute(
                "AllGather", mybir.AluOpType.bypass,
                replica_groups=[list(range(NCORES))],
                ins=[ag_in.opt()], outs=[ag_out.opt()])

            with (
                tc.tile_pool(name="gpsum", bufs=2, space="PSUM") as gps,
                tc.tile_pool(name="cpsum", bufs=2, space="PSUM") as cps,
                tc.tile_pool(name="step", bufs=2) as sp,
            ):
                # per step-pair banks, m-major so the x-matmul dsts are
                # plain contiguous: col = 128*m + 64*s + b
                g_banks = [gps.tile([128, 512], f32, tag="gb", name=f"gb{i}")
                           for i in range(2)]
                c_banks = [cps.tile([128, 256], f32, tag="cb", name=f"cb{i}")
                           for i in range(2)]

                def gates_x(p):
                    """Gate-bank x/bias/mask matmuls for step pair p.

                    All start=True mains first: a small-K accumulate followed
                    by a start=True matmul to the same bank loses its
                    contribution on this stack, so the bias/mask extras go
                    last."""
                    gb = g_banks[p % 2]
                    cs = slice(128 * p, 128 * p + 128)
                    # single start=True opener for the whole bank: multiple
                    # open accumulation groups per bank corrupt each other
                    nc.tensor.matmul(gb[:], id_bf[:], zer512[:],
                                     start=True, stop=False)
                    for m in range(4):
                        for k in range(2):
                            nc.tensor.matmul(
                                gb[:, 128 * m:128 * m + 128],
                                wxg_s[k][:, 128 * m:128 * m + 128],
                                xT[k][:, cs], start=False, stop=False)
                    for m in range(4):
                        nc.tensor.matmul(
                            gb[:, 128 * m:128 * m + 128],
                            wxg_e[:, 128 * m:128 * m + 128],
                            xE[:, cs], start=False, stop=False)

                def cand_x(p):
                    """Cand-bank x/bias matmuls for step pair p."""
                    cb = c_banks[p % 2]
                    cs = slice(128 * p, 128 * p + 128)
                    nc.tensor.matmul(cb[:], id_bf[:], zer512[:, 0:256],
                                     start=True, stop=False)
                    for m in range(2):
                        for k in range(2):
                            nc.tensor.matmul(
                                cb[:, 128 * m:128 * m + 128],
                                wxc_s[k][:, 128 * m:128 * m + 128],
                                xT[k][:, cs], start=False, stop=False)
                    for m in range(2):
                        nc.tensor.matmul(
                            cb[:, 128 * m:128 * m + 128],
                            wxc_e[:, 128 * m:128 * m + 128],
                            xE[:, cs], start=False, stop=False)

                gates_x(0)
                cand_x(0)
                # step 0 has no recurrent gate matmuls (h0 = 0): close its
                # accumulation groups with zero contributions
                g0 = g_banks[0]
                for m in range(4):
                    for k in range(2):
                        nc.tensor.matmul(
                            g0[:, 128 * m:128 * m + 64],
                            whg_s[k][:, 128 * m:128 * m + 128],
                            zer_bf[:, 64 * k:64 * k + 64],
                            start=False, stop=(k == 1))
                gates_x(1)
                cand_x(1)

                h_f = pp.tile([128, 128], f32, tag="hf", name="hf", bufs=2)
                nc.vector.memset(h_f[:], 0.0)

                for t in range(T):
                    gb = g_banks[(t // 2) % 2]
                    cb = c_banks[(t // 2) % 2]
                    s = t % 2
                    gv = gb[:].rearrange("q (m s b) -> q m s b", m=4, s=2,
                                         b=BSH)
                    cv = cb[:].rearrange("q (m s b) -> q m s b", m=2, s=2,
                                         b=BSH)

                    sig_r = sp.tile([128, 128], f32, tag="sigr", name="sigr")
                    nc.scalar.activation(
                        sig_r[:].rearrange("q (m b) -> q m b", m=2, b=BSH),
                        gv[:, 0:2, s, :], AF.Sigmoid)
                    sig_z = sp.tile([128, 128], f32, tag="sigz", name="sigz")
                    nc.scalar.activation(
                        sig_z[:].rearrange("q (m b) -> q m b", m=2, b=BSH),
                        gv[:, 2:4, s, :], AF.Sigmoid)
                    omz = sp.tile([128, 128], f32, tag="omz", name="omz")
                    nc.scalar.activation(
                        omz[:].rearrange("q (m b) -> q m b", m=2, b=BSH),
                        gv[:, 2:4, s, :], AF.Sigmoid, scale=-1.0)

                    rh_bf = sp.tile([128, 128], bf16, tag="rh", name="rh")
                    nc.vector.tensor_mul(rh_bf[:], sig_r[:], h_f[:])
                    zh_bf = sp.tile([128, 128], bf16, tag="zh", name="zh")
                    nc.vector.tensor_mul(zh_bf[:], sig_z[:], h_f[:])

                    # PE: cand matmuls accumulate onto cb's x-part
                    for m in range(2):
                        for k in range(2):
                            nc.tensor.matmul(
                                cv[:, m, s, :],
                                wch_s[k][:, 128 * m:128 * m + 128],
                                rh_bf[:, 64 * k:64 * k + 64],
                                start=False, stop=(k == 1))

                    c_f = sp.tile([128, 128], f32, tag="ct", name="ct")
                    nc.scalar.activation(
                        c_f[:].rearrange("q (m b) -> q m b", m=2, b=BSH),
                        cv[:, :, s, :], AF.Tanh)
                    oc_bf = sp.tile([128, 128], bf16, tag="oc", name="oc")
                    nc.vector.tensor_mul(oc_bf[:], omz[:], c_f[:])
                    h_new = pp.tile([128, 128], f32, tag="hf", name="hf",
                                    bufs=2)
                    nc.vector.tensor_add(h_new[:], zh_bf[:], oc_bf[:])

                    if t + 1 < T:
                        gbn = g_banks[((t + 1) // 2) % 2]
                        sn = 64 * ((t + 1) % 2)
                        for m in range(4):
                            for k in range(2):
                                nc.tensor.matmul(
                                    gbn[:, 128 * m + sn:128 * m + sn + 64],
                                    whg_s[k][:, 128 * m:128 * m + 128],
                                    zh_bf[:, 64 * k:64 * k + 64],
                                    start=False, stop=False)
                        # pair (t+3)//2's x-matmuls: banks' previous readers
                        # (this iteration's sigmoids/tanh) are emitted above,
                        # so the overwrite orders correctly; cand_x fits the
                        # zh->oc PE gap, gates_x the oc->next-cand gap
                        if t % 2 == 1 and (t + 3) // 2 < NPAIR:
                            cand_x((t + 3) // 2)
                        for m in range(4):
                            for k in range(2):
                                nc.tensor.matmul(
                                    gbn[:, 128 * m + sn:128 * m + sn + 64],
                                    whg_s[k][:, 128 * m:128 * m + 128],
                                    oc_bf[:, 64 * k:64 * k + 64],
                                    start=False, stop=(k == 1))
                        if t % 2 == 1 and (t + 3) // 2 < NPAIR:
                            gates_x((t + 3) // 2)
                    h_f = h_new

                h_bf = pp.tile([128, 128], bf16, tag="hbf", name="hbf")
                nc.vector.tensor_copy(h_bf[:], h_f[:])
                nc.scalar.dma_start(hdbg[:], h_bf[:])

            # ---- exchange encodings (collective path is warm now) ----
            nc.gpsimd.dma_start(ag_in[:], h_bf[:])
            nc.gpsimd.collective_compute(
                "AllGather", mybir.AluOpType.bypass,
                replica_groups=[list(range(NCORES))],
                ins=[ag_in.opt()], outs=[ag_out.opt()])

            # per-core q slice: rows [32*core, 32*core+32) live on gathered
            # block core//2, batch-half core%2 -> predicated static DMAs,
            # spread across engine queues
            qloc = pp.tile([128, 2 * IBLK], bf16, tag="qloc", name="qloc")  # [p, c*32+b]
            qeng = [nc.sync, nc.scalar, nc.gpsimd]
            pids = [e.partition_id() for e in qeng]
            for co in range(NCORES):
                e = qeng[co % 3]
                src_v = ag_out[co // 2].rearrange(
                    "p (c h b) -> p c h b", c=2, h=2, b=32)
                e.dma_start(
                    qloc[:].rearrange("p (c b) -> p c b", c=2, b=32),
                    src_v[:, :, co % 2, :], cond=(pids[co % 3] == co))
            # rT[p, 256*c + 64*k + b] <- ag_out[4+k, p, 64*c + b]: one 3D DMA per c
            rT = pp.tile([128, 2 * B], bf16, tag="rT", name="rT")
            for c in range(2):
                src = ag_out[4:8].rearrange("k p (c b) -> p c k b", c=2, b=64)
                [nc.gpsimd, nc.scalar][c].dma_start(
                    rT[:, 256 * c:256 * c + 256].rearrange(
                        "p (k b) -> p k b", k=4, b=64),
                    src[:, c])

            # fused K=4 outer-product operands, one MM per (i-pair, m):
            # lhs4 rows [wd; q1_even; wd; q1_odd], rhs4 rows
            # [dist_even 0; ones 0; 0 dist_odd; 0 ones] per 512-col block
            lhs4 = pp.tile([4, IBLK // 2 * D_HID], bf16, tag="lhs4",
                           name="lhs4")
            nc.sync.dma_start(lhs4[0:1, :], wdt[:])
            nc.sync.dma_start(lhs4[2:3, :], wdt[:])
            rhs4 = pp.tile([4, IBLK * B], bf16, tag="rhs4", name="rhs4")
            nc.sync.dma_start(rhs4[:], rhsb[:])
            q1 = pp.tile([IBLK, D_HID], bf16, tag="q1", name="q1")
            dist = pp.tile([IBLK, B], bf16, tag="dist", name="dist")
            r1tb = pp.tile([128, 2 * B], f32, tag="r1tb", name="r1tb")
            r1tb2 = pp.tile([128, 4 * B], bf16, tag="r1tb2", name="r1tb2")

            with tc.tile_pool(name="prep", bufs=2, space="PSUM") as sps:
                # Q1 rows for my i's: [32, 256] bf16
                ps = sps.tile([IBLK, D_HID], f32, tag="sps", name="sps")
                for c in range(2):
                    nc.tensor.matmul(ps[:], qloc[:, 32 * c:32 * c + 32],
                                     w1q_s[c][:], start=(c == 0), stop=(c == 1))
                nc.scalar.activation(q1[:], ps[:], AF.Copy, bias=0.0)
                nc.sync.dma_start(lhs4[1:2, :], q1[0:16, :])
                nc.scalar.dma_start(lhs4[3:4, :], q1[16:32, :])

                # dist rows for my i's: [32, 256] bf16
                ps2 = sps.tile([IBLK, B], f32, tag="sps", name="sps")
                for c in range(2):
                    nc.tensor.matmul(ps2[:], qloc[:, 32 * c:32 * c + 32],
                                     rT[:, 256 * c:256 * c + 256],
                                     start=(c == 0), stop=(c == 1))
                nc.scalar.activation(dist[:], ps2[:], AF.Copy, bias=0.0)
                nc.sync.dma_start(
                    rhs4[0:1, :].rearrange("o (p ii j) -> o p ii j",
                                           p=IBLK // 2, ii=2, j=B)[:, :, 0, :],
                    dist[0:16, :])
                nc.scalar.dma_start(
                    rhs4[2:3, :].rearrange("o (p ii j) -> o p ii j",
                                           p=IBLK // 2, ii=2, j=B)[:, :, 1, :],
                    dist[16:32, :])

                # R1T + b1: [128, m*256 + j] f32
                for m in range(2):
                    ps3 = sps.tile([128, B], f32, tag="sps", name="sps")
                    for k in range(2):
                        nc.tensor.matmul(ps3[:],
                                         w1r_s[k][:, 128 * m:128 * m + 128],
                                         rT[:, 256 * k:256 * k + 256],
                                         start=(k == 0), stop=(k == 1))
                    nc.scalar.activation(r1tb[:, 256 * m:256 * m + 256], ps3[:],
                                         AF.Identity, bias=b1_s[:, m:m + 1])

                # duplicate per pair member: [128, m*512 + ii*256 + j] bf16
                r2v = r1tb2[:].rearrange("p (m ii j) -> p m ii j", m=2, ii=2,
                                         j=B)
                for ii in range(2):
                    nc.vector.tensor_copy(
                        r2v[:, :, ii, :],
                        r1tb[:].rearrange("p (m j) -> p m j", m=2, j=B))

            with (
                tc.tile_pool(name="hpsum", bufs=2, space="PSUM") as hps,
                tc.tile_pool(name="lpsum", bufs=4, space="PSUM") as lps,
                tc.tile_pool(name="cls", bufs=4) as cp,
            ):
                # W2 outputs: 3 pr per PSUM bank at partition slots 0/32/64;
                # 4 banks recycled (copied out after their 3rd pr)
                lpb = [lps.tile([66, 2 * B], f32, tag="lp", name=f"lp{i}")
                       for i in range(4)]
                out_sb = pp.tile([66, 6 * 512], f32, tag="outsb",
                                 name="outsb")
                gelu_af = AF.Tanh if sim_gelu else AF.Gelu_apprx_tanh
                h1s = {}
                NPR = IBLK // 2

                def emit_w2(pr):
                    ib, sl = divmod(pr, 3)
                    h1 = h1s.pop(pr)
                    for k in range(2):
                        nc.tensor.matmul(
                            lpb[ib % 4][32 * sl:32 * sl + 2, :], w2_s[k][:],
                            h1[:, 512 * k:512 * k + 512],
                            start=(k == 0), stop=(k == 1))
                    if sl == 2 or pr == NPR - 1:
                        nc.vector.tensor_copy(
                            out_sb[:, 512 * ib:512 * ib + 512],
                            lpb[ib % 4][:])

                for pr in range(NPR + 2):
                    if pr < NPR:
                        # h1 pair tile: col = 512*m + 256*ii + j (ii = i in pair)
                        h1 = cp.tile([128, 4 * B], bf16, tag="h1", name="h1")
                        h1s[pr] = h1
                        # m=0: r1 added on DVE; m=1: r1 via identity-MM in PSUM
                        h_ps = hps.tile([128, 2 * B], f32, tag="hps",
                                        name="hps")
                        nc.tensor.matmul(
                            h_ps[:], lhs4[0:4, D_HID * pr:D_HID * pr + 128],
                            rhs4[0:4, 2 * B * pr:2 * B * pr + 2 * B],
                            start=True, stop=True)
                        h_ps2 = hps.tile([128, 2 * B], f32, tag="hps2",
                                         name="hps2")
                        nc.tensor.matmul(
                            h_ps2[:],
                            lhs4[0:4, D_HID * pr + 128:D_HID * pr + 256],
                            rhs4[0:4, 2 * B * pr:2 * B * pr + 2 * B],
                            start=True, stop=False)
                        nc.tensor.matmul(h_ps2[:], id_bf[:],
                                         r1tb2[:, 512:1024],
                                         start=False, stop=True)
                    if pr >= 2:
                        emit_w2(pr - 2)
                    if pr < NPR:
                        h1p = cp.tile([128, 2 * B], f32, tag="h1p", name="h1p")
                        nc.vector.tensor_add(h1p[:], h_ps[:], r1tb2[:, 0:512])
                        nc.scalar.activation(h1[:, 0:512], h1p[:], gelu_af)
                        nc.scalar.activation(h1[:, 512:1024], h_ps2[:],
                                             gelu_af)

                for sl in range(3):
                    [nc.sync, nc.scalar, nc.gpsimd][sl].dma_start(
                        out[2 * sl:2 * sl + 2, :],
                        out_sb[32 * sl:32 * sl + 2, :])

    nc.compile()
    return nc


def _rhs_base():
    """[4, IBLK*B] pattern: per 512-col pair-block rows are
    [0,0],[ones,0],[0,0],[0,ones] - dist blocks get DMA'd in on device."""
    r = np.zeros((4, IBLK * B), dtype=BF16)
    v = r.reshape(4, IBLK // 2, 2, B)
    v[1, :, 0, :] = 1.0
    v[3, :, 1, :] = 1.0
    return r


def _prep_inputs(inputs):
    """Host-side prep: embed+transpose sequences, split weights, per-core maps."""
    emb = inputs["embeddings"]
    in_maps = []
    f32 = np.float32

    # classifier tensors (identical on all cores)
    W1, b1, W2 = (inputs["W1"], inputs["b1"], inputs["W2"])
    common = {
        "w1q": np.ascontiguousarray(W1[:H]).astype(BF16),
        "w1r": np.ascontiguousarray(W1[H + 1:]).astype(BF16),
        "wdt": np.tile(np.ascontiguousarray(W1[H:H + 1]).astype(BF16),
                       (1, IBLK // 2)),
        "rhsb": _rhs_base(),
        "b1": b1.astype(f32),
        "w2": W2.astype(BF16),
        "ident": np.eye(128, dtype=BF16),
    }

    for core in range(NCORES):
        enc = core // NSH
        s = core % NSH
        if enc == 0:
            seqs, lens = inputs["input_queries"], inputs["query_lengths"]
            Wg, bgv, Wc, bcv = (inputs["Wg_q"], inputs["bg_q"],
                                inputs["Wc_q"], inputs["bc_q"])
        else:
            seqs, lens = inputs["input_replies"], inputs["reply_lengths"]
            Wg, bgv, Wc, bcv = (inputs["Wg_r"], inputs["bg_r"],
                                inputs["Wc_r"], inputs["bc_r"])
        rows = slice(BSH * s, BSH * s + BSH)
        xe = emb[seqs[rows]]                       # [64, 40, 256]
        xT = np.transpose(xe, (2, 1, 0)).reshape(E, BT)  # col = t*64+b
        lmask = (np.arange(T)[:, None] >= lens[rows][None, :]) \
            .astype(f32).reshape(1, BT)
        ones = np.ones((1, BT), f32)
        xembT = np.concatenate([xT, ones, lmask], axis=0).astype(BF16)

        mask_row = np.concatenate([np.zeros(H, f32), np.full(H, 30.0, f32)])
        wxg = np.concatenate([Wg[:E], bgv[None, :], mask_row[None, :]],
                             axis=0).astype(BF16)
        wxc = np.concatenate([Wc[:E], bcv[None, :]], axis=0).astype(BF16)

        m = {
            "xembT": xembT,
            "whg": np.ascontiguousarray(Wg[E:]).astype(BF16),
            "wxg": wxg,
            "wch": np.ascontiguousarray(Wc[E:]).astype(BF16),
            "wxc": wxc,
        }
        m.update(common)
        in_maps.append(m)
    return in_maps


def run_cores(in_maps, trace=False):
    from concourse.bass_utils import run_bass_kernel_spmd
    from concourse.bass_interp import get_hw_module

    if "nc" not in _cache:
        _cache["nc"] = _build()
    nc = _cache["nc"]
    old = nc.m
    nc.m = _cache.setdefault("hwm", get_hw_module(nc.m))
    try:
        res = run_bass_kernel_spmd(nc, in_maps, core_ids=list(range(NCORES)),
                                   trace=trace)
    finally:
        nc.m = old
    return res


def kernel(**inputs):
    in_maps = _prep_inputs(inputs)
    res = run_cores(in_maps)
    b2 = inputs["b2"].astype(np.float32)
    logits = np.zeros((B, B, 2), np.float32)
    for core in range(NCORES):
        o = res.results[core]["out"]               # [6, 3072]
        ov = o.reshape(3, 2, 6, 2, B)              # [sl, o, ib, ii, j]
        for pr in range(IBLK // 2):
            ib, sl = divmod(pr, 3)
            for ii in range(2):
                logits[IBLK * core + 16 * ii + pr] = \
                    ov[sl, :, ib, ii, :].T         # [j, o]
    logits += b2
    pos = logits[np.arange(B), np.arange(B)]
    qi, ri = np.nonzero(~np.eye(B, dtype=bool))
    neg = logits[qi, ri]
    return np.concatenate([pos, neg], axis=0).astype(np.float32)


if __name__ == "__main__":
    _build()
    print("build OK")
